# revision 13
# baseline (speedup 1.0000x reference)
"""MoE grouped-GEMM expert FFN (SwiGLU) for Trainium2, 8-core expert parallelism.

Contract: kernel(**inputs) takes FULL unsharded inputs, returns FULL output.

Strategy:
  - Host-side routing: tokens are contiguous per expert; split expert groups
    into chunks, band-assign chunks across 8 cores with an identical
    segment-capacity structure on every core (SPMD: one Bass program).
  - Per core, per segment: local GEMM1 (x @ w1w3) -> SwiGLU -> GEMM2 (h @ w2).
  - Host-side combine: scatter per-core output rows back to full output.

Matmul dtype is configurable (MM_DT): float32r runs at full PE rate with
~2.5e-4 rel err; float16 additionally halves DMA bytes and enables fast
weight load, at ~1e-3 rel err. PSUM/silu/output stay fp32 either way.

Layout choices:
  - All device inputs are host-repacked so every DMA loads long contiguous
    rows with few instructions (DMA issue costs ~0.6-1.3us per instruction
    on the sync sequencer; per-engine DMA bandwidth scales with run length).
  - x: packed per token tile as [tile, 128, 8*512] (hidden chunk k on the
    free dim) -> 1 DMA per token tile.
  - w1w3: columns permuted so psum chunk c holds gate[64c:64c+64] on
    partitions 0:64 and up on 64:128 (SwiGLU = partition-slice op); rows
    packed as [S, 4, 128, 2*1408] (k-chunk pairs) -> 4 DMAs per segment.
  - w2: rows packed as [S, 128, 6*1024] (j on free dim; j=5 has 64 valid
    rows) -> 1 DMA per segment.
  - GEMM1 iterates k (contraction) outer / m inner within m-groups of <=4 so
    compute starts after ~1MB of DMA and segment boundaries pipeline.
  - GEMM2 uses h as stationary ([inter, token] slices) and w2 as moving ->
    output lands token-major in PSUM and stores contiguously (via gpsimd
    queue to keep the sync sequencer free for loads).
"""

import numpy as np

import concourse.bacc as bacc
import concourse.mybir as mybir
from concourse import tile
from concourse.bass_utils import run_bass_kernel_spmd

HIDDEN = 1024
INTER = 704
N_EXPERTS = 32
NCORES = 8
KC = HIDDEN // 128  # 8 k-chunks over hidden
MC = (2 * INTER) // 128  # 11 m-chunks over permuted gate|up dim
JC = (INTER + 127) // 128  # 6 j-chunks over inter for GEMM2 (last is 64 rows)
TT = 512  # token tile (moving free dim)
M_GROUPS = [(0, 4), (4, 8), (8, 11)]  # m-ranges; <=4 psum banks live at once

f32 = mybir.dt.float32

# Matmul input dtype: mybir.dt.float32r (safer numerics) or float16 (faster).
MM_DT = mybir.dt.float32r
NP_DT = np.float32
ESZ = 4  # element size of MM_DT in bytes


def set_dtype(name):
    global MM_DT, NP_DT, ESZ
    if name == "f32r":
        MM_DT, NP_DT, ESZ = mybir.dt.float32r, np.float32, 4
    elif name == "f16":
        MM_DT, NP_DT, ESZ = mybir.dt.float16, np.float16, 2
    elif name == "bf16":
        MM_DT, NP_DT, ESZ = mybir.dt.bfloat16, np.float32, 2  # cast via jax-free trick
    else:
        raise ValueError(name)


# Column permutation of w1w3's last dim (2*INTER): chunk c of 128 holds
# gate[64c:64c+64] then up[64c:64c+64].
_PERM = np.empty(2 * INTER, dtype=np.int64)
for _c in range(MC):
    _PERM[128 * _c : 128 * _c + 64] = np.arange(64 * _c, 64 * _c + 64)
    _PERM[128 * _c + 64 : 128 * _c + 128] = INTER + np.arange(64 * _c, 64 * _c + 64)


def _to_np_dt(a):
    """Cast fp32 array to the host dtype for MM_DT."""
    if MM_DT == mybir.dt.bfloat16:
        b = np.asarray(a, dtype=np.float32).copy()
        v = b.view(np.uint32)
        v += 0x8000  # round-to-nearest-even-ish
        v &= 0xFFFF0000
        return b
    return np.asarray(a, dtype=NP_DT)


def _make_chunks(counts, starts, tmax):
    chunks = []  # (n, expert, tok_start)
    for e in range(N_EXPERTS):
        n = int(counts[e])
        a = int(starts[e])
        if n <= 0:
            continue
        nparts = -(-n // tmax)
        base, rem = divmod(n, nparts)
        off = 0
        for p in range(nparts):
            ln = base + (1 if p < rem else 0)
            if ln > 0:
                chunks.append((ln, e, a + off))
                off += ln
    return chunks


def _plan(counts):
    """Balance (expert, token-chunk) pieces across NCORES cores.

    Chunks are sorted by size and dealt in bands of 8 (one per core): slot s
    capacity = the largest chunk in band s, which minimizes total capacity
    for a given chunk multiset. The split threshold trades segment count
    (weight DMA traffic) against padding (PE + activation traffic).
    """
    starts = np.zeros(N_EXPERTS, dtype=np.int64)
    np.cumsum(counts[:-1], out=starts[1:])

    lw = 210e-9 if ESZ == 4 else 115e-9  # per-pair weight-load floor
    w_seg = (HIDDEN * 2 * INTER + INTER * HIDDEN) * ESZ

    best = None
    for tmax in (4096, 2048, 1536, 1024, *range(256, 1025, 16)):
        chunks = _make_chunks(counts, starts, max(1, tmax))
        if not chunks:
            chunks = [(0, None, 0)]
        chunks.sort(key=lambda c: -c[0])
        S = -(-len(chunks) // NCORES)
        caps = []
        for s in range(S):
            band = chunks[NCORES * s : NCORES * (s + 1)]
            caps.append(max(16, ((band[0][0] + 15) // 16) * 16))
        cap_total = sum(caps)
        dma_t = (S * w_seg + cap_total * HIDDEN * (ESZ + 4)) / 300e9
        pe_t = 0.0
        for C in caps:
            for t0 in range(0, C, TT):
                tt = min(TT, C - t0)
                pe_t += 88 * max(lw, tt * 0.4267e-9)
                pe_t += -(-tt // 128) * 12 * max(lw, 213e-9)
        score = max(dma_t, pe_t) + 0.2 * min(dma_t, pe_t)
        if best is None or score < best[0]:
            best = (score, chunks, S, caps)

    _, chunks, S, caps = best
    offs = np.concatenate([[0], np.cumsum(caps)[:-1]]).astype(np.int64)
    cap_total = int(sum(caps))

    assign = [[] for _ in range(NCORES)]
    for s in range(S):
        band = chunks[NCORES * s : NCORES * (s + 1)]
        for c in range(NCORES):
            if c < len(band):
                n, e, a = band[c]
                assign[c].append((e, a, n))
            else:
                assign[c].append((None, 0, 0))
    return assign, caps, offs, cap_total


def _tiles_of(caps):
    """Token tiles as (segment, t0, tt) in execution order."""
    out = []
    for s, C in enumerate(caps):
        for t0 in range(0, C, TT):
            out.append((s, t0, min(TT, C - t0)))
    return out


def _build(S, caps, cap_total):
    """Build the SPMD Bass program for one core's segment structure."""
    nc = bacc.Bacc("TRN2", target_bir_lowering=False, debug=False, num_devices=NCORES)

    tiles = _tiles_of(caps)
    NT = len(tiles)
    offs = np.concatenate([[0], np.cumsum(caps)[:-1]]).astype(np.int64)

    xt_d = nc.declare_dram_parameter("xt", [NT, 128, KC * TT], MM_DT, isOutput=False)
    w13_d = nc.declare_dram_parameter(
        "w13", [S, 4, 128, 2 * 2 * INTER], MM_DT, isOutput=False
    )
    w2_d = nc.declare_dram_parameter(
        "w2", [S, 128, JC * HIDDEN], MM_DT, isOutput=False
    )
    out_d = nc.declare_dram_parameter("out", [cap_total, HIDDEN], f32, isOutput=True)

    # SBUF pool sizing: slots scale with ESZ; keep total under ~23MB.
    big = ESZ == 4
    w13_bufs = 6 if big else 12
    w2_bufs = 2 if big else 3
    xt_bufs = 3 if big else 4

    with tile.TileContext(nc) as tc:
        with (
            tc.tile_pool(name="w13p", bufs=w13_bufs) as w13p,
            tc.tile_pool(name="w2p", bufs=w2_bufs) as w2p,
            tc.tile_pool(name="xtp", bufs=xt_bufs) as xtp,
            tc.tile_pool(name="hp", bufs=7) as hp,
            tc.tile_pool(name="sgp", bufs=3) as sgp,
            tc.tile_pool(name="outp", bufs=3) as outp,
            tc.tile_pool(name="ps1", bufs=4, space="PSUM") as ps1,
            tc.tile_pool(name="ps2", bufs=2, space="PSUM") as ps2,
        ):
            # HAM warmup: the PE clock sits at 1.2GHz until ~3.4us of
            # sustained matmul activity. Run throwaway matmuls on a zeroed
            # tile while the first real DMAs are still in flight so the real
            # matmul stream starts at 2.4GHz.
            warm_sb = sgp.tile([128, 128], MM_DT, tag="warm", name="warm_sb")
            nc.vector.memset(warm_sb[:], 0.0)
            warm_ps = ps1.tile([128, 128], f32, tag="pg", name="warm_ps",
                               padded_shape=[128, TT])
            for _w in range(56):
                nc.tensor.matmul(
                    warm_ps[:, 0:128],
                    warm_sb[:, 0:128],
                    warm_sb[:, 0:128],
                    start=True,
                    stop=True,
                )

            tix = 0
            for s in range(S):
                C = caps[s]
                off = int(offs[s])

                # For the first segment, issue the first token tile's xt DMA
                # ahead of the weights: the DMA queues drain roughly in issue
                # order, and the first matmul needs (xt, w13 pair 0).
                xt_first = None
                if s == 0:
                    tt0 = min(TT, C)
                    xt_first = xtp.tile([128, KC * tt0], MM_DT, tag="xtt",
                                        name="xtt0",
                                        padded_shape=[128, KC * TT])
                    nc.sync.dma_start(out=xt_first[:], in_=xt_d[0, :, 0 : KC * tt0])

                # Weights for this segment, in first-use (k) order.
                w13_t = []
                for kp in range(4):
                    w13t = w13p.tile([128, 2 * 2 * INTER], MM_DT, tag="w13t",
                                     name=f"w13t{s}_{kp}")
                    nc.sync.dma_start(out=w13t[:], in_=w13_d[s, kp])
                    w13_t.append(w13t)
                w2t = w2p.tile([128, JC * HIDDEN], MM_DT, tag="w2t", name=f"w2t{s}")
                nc.sync.dma_start(out=w2t[:], in_=w2_d[s])

                def w13_ap(k, m):
                    base = (k % 2) * 2 * INTER + 128 * m
                    return w13_t[k // 2][:, base : base + 128]

                def w2_ap(j, nn):
                    jw = min(128, INTER - 128 * j)
                    base = j * HIDDEN + 512 * nn
                    return w2t[0:jw, base : base + 512]

                for t0 in range(0, C, TT):
                    tt = min(TT, C - t0)
                    if t0 == 0 and xt_first is not None:
                        xt_tile = xt_first
                    else:
                        xt_tile = xtp.tile([128, KC * tt], MM_DT, tag="xtt",
                                           name=f"xtt{tix}",
                                           padded_shape=[128, KC * TT])
                        nc.sync.dma_start(
                            out=xt_tile[:], in_=xt_d[tix, :, 0 : KC * tt]
                        )

                    def xt_ap(k):
                        return xt_tile[:, k * tt : (k + 1) * tt]

                    h_t = []
                    for j in range(JC):
                        jw = min(128, INTER - 128 * j)
                        ht = hp.tile([jw, tt], MM_DT, tag="ht", name=f"ht{tix}_{j}",
                                     padded_shape=[128, TT])
                        h_t.append(ht)

                    # GEMM1: k-outer within m-groups of <=4 psum tiles.
                    for m_lo, m_hi in M_GROUPS:
                        pgs = {}
                        for m in range(m_lo, m_hi):
                            pgs[m] = ps1.tile([128, tt], f32, tag="pg",
                                              name=f"pg{m}",
                                              padded_shape=[128, TT])
                        for k in range(KC):
                            for m in range(m_lo, m_hi):
                                nc.tensor.matmul(
                                    pgs[m][:],
                                    w13_ap(k, m),
                                    xt_ap(k),
                                    start=(k == 0),
                                    stop=(k == KC - 1),
                                )
                        for m in range(m_lo, m_hi):
                            sg = sgp.tile([64, tt], f32, tag="sg", name=f"sg{m}",
                                          padded_shape=[64, TT])
                            nc.scalar.activation(
                                sg[:], pgs[m][0:64, :],
                                mybir.ActivationFunctionType.Silu,
                            )
                            j, half = divmod(m, 2)
                            nc.vector.tensor_mul(
                                h_t[j][64 * half : 64 * half + 64, :],
                                sg[:],
                                pgs[m][64:128, :],
                            )

                    # GEMM2: h stationary, w2 moving; token-major output.
                    for tc0 in range(0, tt, 128):
                        tw = min(128, tt - tc0)
                        po = ps2.tile([tw, HIDDEN], f32, tag="po", name="po",
                                      padded_shape=[128, HIDDEN])
                        for j in range(JC):
                            for nn in range(HIDDEN // 512):
                                nc.tensor.matmul(
                                    po[:, 512 * nn : 512 * (nn + 1)],
                                    h_t[j][:, tc0 : tc0 + tw],
                                    w2_ap(j, nn),
                                    start=(j == 0),
                                    stop=(j == JC - 1),
                                )
                        ob = outp.tile([tw, HIDDEN], f32, tag="ob", name="ob",
                                       padded_shape=[128, HIDDEN])
                        nc.vector.tensor_copy(ob[:], po[:])
                        nc.sync.dma_start(
                            out=out_d[off + t0 + tc0 : off + t0 + tc0 + tw, :],
                            in_=ob[:],
                        )
                    tix += 1

    nc.compile()
    return nc


_BUILD_CACHE = {}


def _get_program(S, caps, cap_total):
    key = (S, tuple(caps), str(MM_DT))
    if key not in _BUILD_CACHE:
        _BUILD_CACHE[key] = _build(S, caps, cap_total)
    return _BUILD_CACHE[key]


def _pack_inputs(x, assign, caps, offs, cap_total, w13_perm, w2):
    """Build per-core input dicts matching the device layouts."""
    tiles = _tiles_of(caps)
    NT = len(tiles)
    S = len(caps)
    in_maps = []
    for c in range(NCORES):
        xt_c = np.zeros((HIDDEN, cap_total), dtype=NP_DT)
        w13_c = np.zeros((S, 4, 128, 2 * 2 * INTER), dtype=NP_DT)
        w2_c = np.zeros((S, 128, JC * HIDDEN), dtype=NP_DT)
        for s, (e, a, n) in enumerate(assign[c]):
            if e is None or n <= 0:
                continue
            o = int(offs[s])
            xt_c[:, o : o + n] = _to_np_dt(x[a : a + n, :]).T
            # w13: [1024, 1408] -> [4, 2, 128, 1408] -> [4, 128, 2*1408]
            w13_c[s] = (
                w13_perm["w13"][e]
                .reshape(4, 2, 128, 2 * INTER)
                .transpose(0, 2, 1, 3)
                .reshape(4, 128, 2 * 2 * INTER)
            )
            # w2: pad [704,1024] -> [768,1024] -> [6,128,1024] -> [128, 6*1024]
            w2_c[s] = w13_perm["w2"][e]
        # xt: per token tile [1024, tt] -> [8, 128, tt] -> [128, 8*tt]
        xt_pack = np.zeros((NT, 128, KC * TT), dtype=NP_DT)
        for tix, (s, t0, tt) in enumerate(tiles):
            o = int(offs[s])
            blk = xt_c[:, o + t0 : o + t0 + tt]  # [1024, tt]
            xt_pack[tix, :, 0 : KC * tt] = (
                blk.reshape(KC, 128, tt).transpose(1, 0, 2).reshape(128, KC * tt)
            )
        in_maps.append({"xt": xt_pack, "w13": w13_c, "w2": w2_c})
    return in_maps


def _prep_weights(w1w3, w2):
    """Permute/pack weights once (shared across cores)."""
    w13_perm = _to_np_dt(w1w3[:, :, _PERM])  # [E, HIDDEN, 2*INTER]
    w2p_all = np.zeros((N_EXPERTS, 768, HIDDEN), dtype=NP_DT)
    w2p_all[:, :INTER] = _to_np_dt(w2)
    w2_pack = (
        w2p_all.reshape(N_EXPERTS, JC, 128, HIDDEN)
        .transpose(0, 2, 1, 3)
        .reshape(N_EXPERTS, 128, JC * HIDDEN)
    )
    return {"w13": w13_perm, "w2": w2_pack}


def _run(x, tokens_per_expert, w1w3, w2, trace=False):
    x = np.ascontiguousarray(np.asarray(x, dtype=np.float32))
    counts = np.asarray(tokens_per_expert, dtype=np.int64).copy()
    w1w3 = np.asarray(w1w3, dtype=np.float32)
    w2 = np.asarray(w2, dtype=np.float32)

    T = x.shape[0]
    # Clip group sizes like ragged_dot: groups are consecutive; anything
    # beyond T is out of range.
    counts = np.maximum(counts, 0)
    cum = np.cumsum(counts)
    over = cum > T
    if over.any():
        first = int(np.argmax(over))
        prev = int(cum[first - 1]) if first > 0 else 0
        counts[first] = T - prev
        counts[first + 1 :] = 0

    assign, caps, offs, cap_total = _plan(counts)
    S = len(caps)
    nc = _get_program(S, caps, cap_total)

    packed_w = _prep_weights(w1w3, w2)
    in_maps = _pack_inputs(x, assign, caps, offs, cap_total, packed_w, w2)

    extra = {}
    if trace:
        import os

        os.makedirs("/tmp/moe_prof", exist_ok=True)
        for f in os.listdir("/tmp/moe_prof"):
            os.unlink(os.path.join("/tmp/moe_prof", f))
        extra["tmpdir"] = "/tmp/moe_prof"
    res = run_bass_kernel_spmd(nc, in_maps, list(range(NCORES)), trace=trace, **extra)

    out_full = np.zeros((T, HIDDEN), dtype=np.float32)
    for c in range(NCORES):
        oc = res.results[c]["out"]
        for s, (e, a, n) in enumerate(assign[c]):
            if e is None or n <= 0:
                continue
            o = int(offs[s])
            out_full[a : a + n, :] = oc[o : o + n, :]
    return out_full, res


def kernel(x, tokens_per_expert, w1w3, w2, decoding=False, **_ignored):
    out, _ = _run(x, tokens_per_expert, w1w3, w2, trace=False)
    return out


# revision 14
# speedup vs baseline: 1.1189x; 1.1189x over previous
"""MoE grouped-GEMM expert FFN (SwiGLU) for Trainium2, 8-core expert parallelism.

Contract: kernel(**inputs) takes FULL unsharded inputs, returns FULL output.

Strategy:
  - Host-side routing: tokens are contiguous per expert; split expert groups
    into chunks, band-assign chunks across 8 cores with an identical
    segment-capacity structure on every core (SPMD: one Bass program).
  - Per core, per segment: local GEMM1 (x @ w1w3) -> SwiGLU -> GEMM2 (h @ w2).
  - Host-side combine: scatter per-core output rows back to full output.

Matmul dtype is configurable (MM_DT): float32r runs at full PE rate with
~2.5e-4 rel err; float16 additionally halves DMA bytes and enables fast
weight load, at ~1e-3 rel err. PSUM/silu/output stay fp32 either way.

Layout choices:
  - All device inputs are host-repacked so every DMA loads long contiguous
    rows with few instructions (DMA issue costs ~0.6-1.3us per instruction
    on the sync sequencer; per-engine DMA bandwidth scales with run length).
  - x: packed per token tile as [tile, 128, 8*512] (hidden chunk k on the
    free dim) -> 1 DMA per token tile.
  - w1w3: columns permuted so psum chunk c holds gate[64c:64c+64] on
    partitions 0:64 and up on 64:128 (SwiGLU = partition-slice op); rows
    packed as [S, 4, 128, 2*1408] (k-chunk pairs) -> 4 DMAs per segment.
  - w2: rows packed as [S, 128, 6*1024] (j on free dim; j=5 has 64 valid
    rows) -> 1 DMA per segment.
  - GEMM1 iterates k (contraction) outer / m inner within m-groups of <=4 so
    compute starts after ~1MB of DMA and segment boundaries pipeline.
  - GEMM2 uses h as stationary ([inter, token] slices) and w2 as moving ->
    output lands token-major in PSUM and stores contiguously (via gpsimd
    queue to keep the sync sequencer free for loads).
"""

import numpy as np

import concourse.bacc as bacc
import concourse.mybir as mybir
from concourse import tile
from concourse.bass_utils import run_bass_kernel_spmd

HIDDEN = 1024
INTER = 704
N_EXPERTS = 32
NCORES = 8
KC = HIDDEN // 128  # 8 k-chunks over hidden
MC = (2 * INTER) // 128  # 11 m-chunks over permuted gate|up dim
JC = (INTER + 127) // 128  # 6 j-chunks over inter for GEMM2 (last is 64 rows)
TT = 512  # token tile (moving free dim)
M_GROUPS = [(0, 4), (4, 8), (8, 11)]  # m-ranges; <=4 psum banks live at once

f32 = mybir.dt.float32

# Matmul input dtype: mybir.dt.float32r (safer numerics) or float16 (faster).
MM_DT = mybir.dt.float32r
NP_DT = np.float32
ESZ = 4  # element size of MM_DT in bytes


def set_dtype(name):
    global MM_DT, NP_DT, ESZ
    if name == "f32r":
        MM_DT, NP_DT, ESZ = mybir.dt.float32r, np.float32, 4
    elif name == "f16":
        MM_DT, NP_DT, ESZ = mybir.dt.float16, np.float16, 2
    elif name == "bf16":
        MM_DT, NP_DT, ESZ = mybir.dt.bfloat16, np.float32, 2  # cast via jax-free trick
    else:
        raise ValueError(name)


# Column permutation of w1w3's last dim (2*INTER): chunk c of 128 holds
# gate[64c:64c+64] then up[64c:64c+64].
_PERM = np.empty(2 * INTER, dtype=np.int64)
for _c in range(MC):
    _PERM[128 * _c : 128 * _c + 64] = np.arange(64 * _c, 64 * _c + 64)
    _PERM[128 * _c + 64 : 128 * _c + 128] = INTER + np.arange(64 * _c, 64 * _c + 64)


def _to_np_dt(a):
    """Cast fp32 array to the host dtype for MM_DT."""
    if MM_DT == mybir.dt.bfloat16:
        b = np.asarray(a, dtype=np.float32).copy()
        v = b.view(np.uint32)
        v += 0x8000  # round-to-nearest-even-ish
        v &= 0xFFFF0000
        return b
    return np.asarray(a, dtype=NP_DT)


def _make_chunks(counts, starts, tmax):
    chunks = []  # (n, expert, tok_start)
    for e in range(N_EXPERTS):
        n = int(counts[e])
        a = int(starts[e])
        if n <= 0:
            continue
        nparts = -(-n // tmax)
        base, rem = divmod(n, nparts)
        off = 0
        for p in range(nparts):
            ln = base + (1 if p < rem else 0)
            if ln > 0:
                chunks.append((ln, e, a + off))
                off += ln
    return chunks


def _plan(counts):
    """Balance (expert, token-chunk) pieces across NCORES cores.

    Chunks are sorted by size and dealt in bands of 8 (one per core): slot s
    capacity = the largest chunk in band s, which minimizes total capacity
    for a given chunk multiset. The split threshold trades segment count
    (weight DMA traffic) against padding (PE + activation traffic).
    """
    starts = np.zeros(N_EXPERTS, dtype=np.int64)
    np.cumsum(counts[:-1], out=starts[1:])

    lw = 210e-9 if ESZ == 4 else 115e-9  # per-pair weight-load floor
    w_seg = (HIDDEN * 2 * INTER + INTER * HIDDEN) * ESZ

    best = None
    for tmax in (4096, 2048, 1536, 1024, *range(256, 1025, 16)):
        chunks = _make_chunks(counts, starts, max(1, tmax))
        if not chunks:
            chunks = [(0, None, 0)]
        chunks.sort(key=lambda c: -c[0])
        S = -(-len(chunks) // NCORES)
        caps = []
        for s in range(S):
            band = chunks[NCORES * s : NCORES * (s + 1)]
            caps.append(max(16, ((band[0][0] + 15) // 16) * 16))
        cap_total = sum(caps)
        dma_t = (S * w_seg + cap_total * HIDDEN * (ESZ + 4)) / 300e9
        pe_t = 0.0
        for C in caps:
            for t0 in range(0, C, TT):
                tt = min(TT, C - t0)
                pe_t += 88 * max(lw, tt * 0.4267e-9)
                pe_t += -(-tt // 128) * 12 * max(lw, 213e-9)
        score = max(dma_t, pe_t) + 0.2 * min(dma_t, pe_t)
        if best is None or score < best[0]:
            best = (score, chunks, S, caps)

    _, chunks, S, caps = best
    offs = np.concatenate([[0], np.cumsum(caps)[:-1]]).astype(np.int64)
    cap_total = int(sum(caps))

    assign = [[] for _ in range(NCORES)]
    for s in range(S):
        band = chunks[NCORES * s : NCORES * (s + 1)]
        for c in range(NCORES):
            if c < len(band):
                n, e, a = band[c]
                assign[c].append((e, a, n))
            else:
                assign[c].append((None, 0, 0))
    return assign, caps, offs, cap_total


def _tiles_of(caps):
    """Token tiles as (segment, t0, tt) in execution order."""
    out = []
    for s, C in enumerate(caps):
        for t0 in range(0, C, TT):
            out.append((s, t0, min(TT, C - t0)))
    return out


def _build(S, caps, cap_total):
    """Build the SPMD Bass program for one core's segment structure."""
    nc = bacc.Bacc("TRN2", target_bir_lowering=False, debug=False, num_devices=NCORES)

    tiles = _tiles_of(caps)
    NT = len(tiles)
    offs = np.concatenate([[0], np.cumsum(caps)[:-1]]).astype(np.int64)

    xt_d = nc.declare_dram_parameter("xt", [NT, 128, KC * TT], MM_DT, isOutput=False)
    w13_d = nc.declare_dram_parameter(
        "w13", [S, 4, 128, 2 * 2 * INTER], MM_DT, isOutput=False
    )
    w2_d = nc.declare_dram_parameter(
        "w2", [S, 128, JC * HIDDEN], MM_DT, isOutput=False
    )
    out_d = nc.declare_dram_parameter("out", [cap_total, HIDDEN], f32, isOutput=True)

    # SBUF pool sizing: slots scale with ESZ; keep total under ~23MB.
    big = ESZ == 4
    w13_bufs = 6 if big else 12
    w2_bufs = 2 if big else 3
    xt_bufs = 3 if big else 4

    with tile.TileContext(nc) as tc:
        with (
            tc.tile_pool(name="w13p", bufs=w13_bufs) as w13p,
            tc.tile_pool(name="w2p", bufs=w2_bufs) as w2p,
            tc.tile_pool(name="xtp", bufs=xt_bufs) as xtp,
            tc.tile_pool(name="hp", bufs=7) as hp,
            tc.tile_pool(name="sgp", bufs=3) as sgp,
            tc.tile_pool(name="outp", bufs=3) as outp,
            tc.tile_pool(name="ps1", bufs=4, space="PSUM") as ps1,
            tc.tile_pool(name="ps2", bufs=2, space="PSUM") as ps2,
        ):
            # HAM warmup: the PE clock sits at 1.2GHz until ~3.4us of
            # sustained matmul activity. Run throwaway matmuls on a zeroed
            # tile while the first real DMAs are still in flight so the real
            # matmul stream starts at 2.4GHz.
            warm_sb = sgp.tile([128, 128], MM_DT, tag="warm", name="warm_sb")
            nc.vector.memset(warm_sb[:], 0.0)
            warm_ps = ps1.tile([128, 128], f32, tag="pg", name="warm_ps",
                               padded_shape=[128, TT])
            for _w in range(56):
                nc.tensor.matmul(
                    warm_ps[:, 0:128],
                    warm_sb[:, 0:128],
                    warm_sb[:, 0:128],
                    start=True,
                    stop=True,
                )

            tix = 0
            for s in range(S):
                C = caps[s]
                off = int(offs[s])

                # For the first segment, issue the first token tile's xt DMA
                # ahead of the weights: the DMA queues drain roughly in issue
                # order, and the first matmul needs (xt, w13 pair 0).
                xt_first = None
                if s == 0:
                    tt0 = min(TT, C)
                    xt_first = xtp.tile([128, KC * tt0], MM_DT, tag="xtt",
                                        name="xtt0",
                                        padded_shape=[128, KC * TT])
                    nc.sync.dma_start(out=xt_first[:], in_=xt_d[0, :, 0 : KC * tt0])

                # Weights for this segment, in first-use (k) order.
                w13_t = []
                for kp in range(4):
                    w13t = w13p.tile([128, 2 * 2 * INTER], MM_DT, tag="w13t",
                                     name=f"w13t{s}_{kp}")
                    nc.sync.dma_start(out=w13t[:], in_=w13_d[s, kp])
                    w13_t.append(w13t)
                w2t = w2p.tile([128, JC * HIDDEN], MM_DT, tag="w2t", name=f"w2t{s}")
                nc.sync.dma_start(out=w2t[:], in_=w2_d[s])

                def w13_ap(k, m):
                    base = (k % 2) * 2 * INTER + 128 * m
                    return w13_t[k // 2][:, base : base + 128]

                def w2_ap(j, nn):
                    jw = min(128, INTER - 128 * j)
                    base = j * HIDDEN + 512 * nn
                    return w2t[0:jw, base : base + 512]

                for t0 in range(0, C, TT):
                    tt = min(TT, C - t0)
                    if t0 == 0 and xt_first is not None:
                        xt_tile = xt_first
                    else:
                        xt_tile = xtp.tile([128, KC * tt], MM_DT, tag="xtt",
                                           name=f"xtt{tix}",
                                           padded_shape=[128, KC * TT])
                        nc.sync.dma_start(
                            out=xt_tile[:], in_=xt_d[tix, :, 0 : KC * tt]
                        )

                    def xt_ap(k):
                        return xt_tile[:, k * tt : (k + 1) * tt]

                    h_t = []
                    for j in range(JC):
                        jw = min(128, INTER - 128 * j)
                        ht = hp.tile([jw, tt], MM_DT, tag="ht", name=f"ht{tix}_{j}",
                                     padded_shape=[128, TT])
                        h_t.append(ht)

                    # GEMM1: k-outer within m-groups of <=4 psum tiles.
                    for m_lo, m_hi in M_GROUPS:
                        pgs = {}
                        for m in range(m_lo, m_hi):
                            pgs[m] = ps1.tile([128, tt], f32, tag="pg",
                                              name=f"pg{m}",
                                              padded_shape=[128, TT])
                        for k in range(KC):
                            for m in range(m_lo, m_hi):
                                nc.tensor.matmul(
                                    pgs[m][:],
                                    w13_ap(k, m),
                                    xt_ap(k),
                                    start=(k == 0),
                                    stop=(k == KC - 1),
                                )
                        for m in range(m_lo, m_hi):
                            sg = sgp.tile([64, tt], f32, tag="sg", name=f"sg{m}",
                                          padded_shape=[64, TT])
                            nc.scalar.activation(
                                sg[:], pgs[m][0:64, :],
                                mybir.ActivationFunctionType.Silu,
                            )
                            j, half = divmod(m, 2)
                            nc.vector.tensor_mul(
                                h_t[j][64 * half : 64 * half + 64, :],
                                sg[:],
                                pgs[m][64:128, :],
                            )

                    # GEMM2: h stationary, w2 moving; token-major output.
                    for tc0 in range(0, tt, 128):
                        tw = min(128, tt - tc0)
                        po = ps2.tile([tw, HIDDEN], f32, tag="po", name="po",
                                      padded_shape=[128, HIDDEN])
                        for j in range(JC):
                            for nn in range(HIDDEN // 512):
                                nc.tensor.matmul(
                                    po[:, 512 * nn : 512 * (nn + 1)],
                                    h_t[j][:, tc0 : tc0 + tw],
                                    w2_ap(j, nn),
                                    start=(j == 0),
                                    stop=(j == JC - 1),
                                )
                        ob = outp.tile([tw, HIDDEN], f32, tag="ob", name="ob",
                                       padded_shape=[128, HIDDEN])
                        nc.vector.tensor_copy(ob[:], po[:])
                        nc.gpsimd.dma_start(
                            out=out_d[off + t0 + tc0 : off + t0 + tc0 + tw, :],
                            in_=ob[:],
                        )
                    tix += 1

    nc.compile()
    return nc


_BUILD_CACHE = {}


def _get_program(S, caps, cap_total):
    key = (S, tuple(caps), str(MM_DT))
    if key not in _BUILD_CACHE:
        _BUILD_CACHE[key] = _build(S, caps, cap_total)
    return _BUILD_CACHE[key]


def _pack_inputs(x, assign, caps, offs, cap_total, w13_perm, w2):
    """Build per-core input dicts matching the device layouts."""
    tiles = _tiles_of(caps)
    NT = len(tiles)
    S = len(caps)
    in_maps = []
    for c in range(NCORES):
        xt_c = np.zeros((HIDDEN, cap_total), dtype=NP_DT)
        w13_c = np.zeros((S, 4, 128, 2 * 2 * INTER), dtype=NP_DT)
        w2_c = np.zeros((S, 128, JC * HIDDEN), dtype=NP_DT)
        for s, (e, a, n) in enumerate(assign[c]):
            if e is None or n <= 0:
                continue
            o = int(offs[s])
            xt_c[:, o : o + n] = _to_np_dt(x[a : a + n, :]).T
            # w13: [1024, 1408] -> [4, 2, 128, 1408] -> [4, 128, 2*1408]
            w13_c[s] = (
                w13_perm["w13"][e]
                .reshape(4, 2, 128, 2 * INTER)
                .transpose(0, 2, 1, 3)
                .reshape(4, 128, 2 * 2 * INTER)
            )
            # w2: pad [704,1024] -> [768,1024] -> [6,128,1024] -> [128, 6*1024]
            w2_c[s] = w13_perm["w2"][e]
        # xt: per token tile [1024, tt] -> [8, 128, tt] -> [128, 8*tt]
        xt_pack = np.zeros((NT, 128, KC * TT), dtype=NP_DT)
        for tix, (s, t0, tt) in enumerate(tiles):
            o = int(offs[s])
            blk = xt_c[:, o + t0 : o + t0 + tt]  # [1024, tt]
            xt_pack[tix, :, 0 : KC * tt] = (
                blk.reshape(KC, 128, tt).transpose(1, 0, 2).reshape(128, KC * tt)
            )
        in_maps.append({"xt": xt_pack, "w13": w13_c, "w2": w2_c})
    return in_maps


def _prep_weights(w1w3, w2):
    """Permute/pack weights once (shared across cores)."""
    w13_perm = _to_np_dt(w1w3[:, :, _PERM])  # [E, HIDDEN, 2*INTER]
    w2p_all = np.zeros((N_EXPERTS, 768, HIDDEN), dtype=NP_DT)
    w2p_all[:, :INTER] = _to_np_dt(w2)
    w2_pack = (
        w2p_all.reshape(N_EXPERTS, JC, 128, HIDDEN)
        .transpose(0, 2, 1, 3)
        .reshape(N_EXPERTS, 128, JC * HIDDEN)
    )
    return {"w13": w13_perm, "w2": w2_pack}


def _run(x, tokens_per_expert, w1w3, w2, trace=False):
    x = np.ascontiguousarray(np.asarray(x, dtype=np.float32))
    counts = np.asarray(tokens_per_expert, dtype=np.int64).copy()
    w1w3 = np.asarray(w1w3, dtype=np.float32)
    w2 = np.asarray(w2, dtype=np.float32)

    T = x.shape[0]
    # Clip group sizes like ragged_dot: groups are consecutive; anything
    # beyond T is out of range.
    counts = np.maximum(counts, 0)
    cum = np.cumsum(counts)
    over = cum > T
    if over.any():
        first = int(np.argmax(over))
        prev = int(cum[first - 1]) if first > 0 else 0
        counts[first] = T - prev
        counts[first + 1 :] = 0

    assign, caps, offs, cap_total = _plan(counts)
    S = len(caps)
    nc = _get_program(S, caps, cap_total)

    packed_w = _prep_weights(w1w3, w2)
    in_maps = _pack_inputs(x, assign, caps, offs, cap_total, packed_w, w2)

    extra = {}
    if trace:
        import os

        os.makedirs("/tmp/moe_prof", exist_ok=True)
        for f in os.listdir("/tmp/moe_prof"):
            os.unlink(os.path.join("/tmp/moe_prof", f))
        extra["tmpdir"] = "/tmp/moe_prof"
    res = run_bass_kernel_spmd(nc, in_maps, list(range(NCORES)), trace=trace, **extra)

    out_full = np.zeros((T, HIDDEN), dtype=np.float32)
    for c in range(NCORES):
        oc = res.results[c]["out"]
        for s, (e, a, n) in enumerate(assign[c]):
            if e is None or n <= 0:
                continue
            o = int(offs[s])
            out_full[a : a + n, :] = oc[o : o + n, :]
    return out_full, res


def kernel(x, tokens_per_expert, w1w3, w2, decoding=False, **_ignored):
    out, _ = _run(x, tokens_per_expert, w1w3, w2, trace=False)
    return out
